# revision 27
# baseline (speedup 1.0000x reference)
"""PixCycleContrastive loss kernel for 8 Trainium2 NeuronCores.

Data-parallel over N=32 images (4 per core). Per image and direction the
[1024,1024] logit block is computed on PE (fp16 inputs, fp32 accumulate),
exp+row-sum on ACT (accum_out), row-max on DVE (fp32), argmax-onehot mask on
GPSIMD, and the "gathered colmax" terms are reduced to per-column counts via a
ones-matmul on PE (sum_i log max1[arg2[i]] == sum_j count2[j] * log max1[j]).
Device returns per-row log-stats + counts; the host does the final (tiny)
linear combine == the scalar all-reduce.
"""

import numpy as np

import concourse.bacc as bacc
import concourse.bass as bass
import concourse.mybir as mybir
import concourse.tile as tile
from concourse.bass_utils import run_bass_kernel_spmd

F32 = mybir.dt.float32
F16 = mybir.dt.bfloat16
AX = mybir.AxisListType
AF = mybir.ActivationFunctionType
ALU = mybir.AluOpType

N_CORES = 8
N, C, H, W = 32, 64, 32, 32
L = H * W  # 1024
M_PER_CORE = N // N_CORES  # 4 images per core
N_TENS = 2 * M_PER_CORE  # 8 input maps per core (rgb/ir x 4 images)
N_DIRS = 2 * M_PER_CORE  # 8 image-directions per core
CHUNK = 128
N_CHUNKS = L // CHUNK  # 8
TEMP = 0.1
EPS = 1e-6


def _body(ctx, tc, rgb, ir, stats_out, counts_out):
    nc = tc.nc

    from contextlib import ExitStack

    const_pool = ctx.enter_context(tc.tile_pool(name="const", bufs=1))
    xn_pool = ctx.enter_context(tc.tile_pool(name="xn", bufs=1))
    stats_pool = ctx.enter_context(tc.tile_pool(name="stats", bufs=1))
    e_pool = ctx.enter_context(tc.tile_pool(name="e", bufs=3))
    mask_pool = ctx.enter_context(tc.tile_pool(name="mask", bufs=3))
    scr_pool = ctx.enter_context(tc.tile_pool(name="scr", bufs=2))

    ones64 = const_pool.tile([C, CHUNK], F16, tag="ones64")
    nc.vector.memset(ones64[:], 1.0)
    ones128 = const_pool.tile([CHUNK, 1], F16, tag="ones128")
    nc.vector.memset(ones128[:], 1.0)
    eps_tile = const_pool.tile([CHUNK, 1], F32, tag="eps")
    nc.vector.memset(eps_tile[:], EPS)

    # Normalized fp16 inputs, all resident, one tile per map so the main loop
    # can start as soon as its two maps are ready. t=2m+{0:rgb,1:ir}.
    xn = [xn_pool.tile([C, L], F16, tag=f"xn{t}", name=f"xn{t}") for t in range(N_TENS)]

    # Main-loop PSUM pools FIRST so they own banks 0-5 and the main loop can
    # start while the norm stage still holds banks 6-7.
    s_psum_pool = ctx.enter_context(tc.tile_pool(name="spsum", bufs=2, space="PSUM"))
    c_psum_pool = ctx.enter_context(tc.tile_pool(name="cpsum", bufs=1, space="PSUM"))

    # ---- Stage 0: load + l2-normalize (over C) each [64, 1024] map ----
    with ExitStack() as norm_ctx:
        x_pool = norm_ctx.enter_context(tc.tile_pool(name="x", bufs=4))
        xsq_pool = norm_ctx.enter_context(tc.tile_pool(name="xsq", bufs=4))
        rn_pool = norm_ctx.enter_context(tc.tile_pool(name="rn", bufs=4))
        norm_psum = norm_ctx.enter_context(
            tc.tile_pool(name="npsum", bufs=1, space="PSUM")
        )
        for t in range(N_TENS):
            m, is_ir = divmod(t, 2)
            src = (ir if is_ir else rgb)[m]
            x = x_pool.tile([C, L], F32, tag="x")
            nc.sync.dma_start(x[:], src)
            xsq = xsq_pool.tile([C, L], F16, tag="xsq")
            nc.scalar.square(xsq[:], x[:])
            # nsq replicated across all 128 output partitions (M=128 ones)
            nsq = norm_psum.tile([C, L], F32, tag="nsq")
            for h in range(2):
                nc.tensor.matmul(
                    nsq[:, h * 512 : (h + 1) * 512],
                    ones64[:, :C],
                    xsq[:, h * 512 : (h + 1) * 512],
                    start=True,
                    stop=True,
                )
            rlog = rn_pool.tile([C, L], F32, tag="rlog")
            nc.scalar.activation(rlog[:], nsq[:], AF.Ln)
            rn = rn_pool.tile([C, L], F32, tag="rn")
            nc.scalar.activation(rn[:], rlog[:], AF.Exp, scale=-0.5)
            nc.gpsimd.tensor_mul(xn[t][:], x[:], rn[:])

    # ---- Stage 1: per image-direction logit chunks + reductions ----
    # dir d = 2m + s: s=0 rows=rgb (lhsT=rgbn, rhs=irn), s=1 rows=ir.
    emax_all = stats_pool.tile([CHUNK, N_DIRS * N_CHUNKS], F32, tag="emax")
    sum_all = stats_pool.tile([CHUNK, N_DIRS * N_CHUNKS], F32, tag="sum")
    counts_all = stats_pool.tile([1, N_DIRS * L], F32, tag="counts")

    for d in range(N_DIRS):
        m, s = divmod(d, 2)
        lhs_t = 2 * m + s  # rgb for s=0, ir for s=1
        rhs_t = 2 * m + (1 - s)
        lhs = xn[lhs_t][:]
        rhs = xn[rhs_t][:]
        cnt = c_psum_pool.tile([1, L], F32, tag="cnt")
        masks = []
        for k in range(N_CHUNKS):
            sp = s_psum_pool.tile([CHUNK, L], F32, tag="sp")
            for h in range(2):
                nc.tensor.matmul(
                    sp[:, h * 512 : (h + 1) * 512],
                    lhs[:, k * CHUNK : (k + 1) * CHUNK],
                    rhs[:, h * 512 : (h + 1) * 512],
                    start=True,
                    stop=True,
                )
            col = d * N_CHUNKS + k
            e = e_pool.tile([CHUNK, L], F32, tag="e")
            nc.scalar.activation(
                e[:], sp[:], AF.Exp, scale=1.0 / TEMP,
                accum_out=sum_all[:, col : col + 1],
            )
            nc.vector.reduce_max(emax_all[:, col : col + 1], e[:], axis=AX.X)
            mask = mask_pool.tile([CHUNK, L], F16, tag="mask")
            nc.vector.tensor_scalar(
                mask[:], e[:], emax_all[:, col : col + 1], None, ALU.is_equal
            )
            masks.append(mask)
            # count-matmul for the PREVIOUS chunk: keeps PE from stalling on
            # this chunk's exp/max/mask chain.
            if k > 0:
                for h in range(2):
                    nc.tensor.matmul(
                        cnt[0:1, h * 512 : (h + 1) * 512],
                        ones128[:],
                        masks[k - 1][:, h * 512 : (h + 1) * 512],
                        start=(k == 1),
                        stop=False,
                    )
        for h in range(2):
            nc.tensor.matmul(
                cnt[0:1, h * 512 : (h + 1) * 512],
                ones128[:],
                masks[N_CHUNKS - 1][:, h * 512 : (h + 1) * 512],
                start=False,
                stop=True,
            )
        nc.scalar.copy(counts_all[0:1, d * L : (d + 1) * L], cnt[0:1, :])

    # ---- Stage 2: logs of maxes and sums, ship stats ----
    logs = stats_pool.tile([CHUNK, 2 * N_DIRS * N_CHUNKS], F32, tag="logs")
    ncols = N_DIRS * N_CHUNKS  # 64
    nc.scalar.activation(logs[:, 0:ncols], emax_all[:], AF.Ln)
    nc.scalar.activation(logs[:, ncols : 2 * ncols], sum_all[:], AF.Ln, bias=eps_tile[:])
    nc.sync.dma_start(stats_out, logs[:])
    nc.sync.dma_start(counts_out.rearrange("d l -> (d l)"), counts_all[:])


_CACHE = {}

# Restrict the ACT table sets the compiler may pick so Exp and Ln resolve to
# the SAME set (natural_log_exp_and_others) -> no table reloads mid-kernel.
_KEEP_SETS = {"natural_log_exp_and_others"}


def _patched_tables(module_arch):
    from concourse.hw_specs import get_activation_tables

    t = get_activation_tables(module_arch)
    return {name: (funcs if name in _KEEP_SETS else set()) for name, funcs in t.items()}


def _build():
    if "nc" in _CACHE:
        return _CACHE["nc"]
    bacc.get_activation_tables = _patched_tables
    nc = bacc.Bacc(
        "TRN2", target_bir_lowering=False, debug=False, num_devices=N_CORES
    )
    rgb = nc.dram_tensor("rgb", [M_PER_CORE, C, L], F32, kind="ExternalInput").ap()
    ir = nc.dram_tensor("ir", [M_PER_CORE, C, L], F32, kind="ExternalInput").ap()
    stats_out = nc.dram_tensor(
        "stats", [CHUNK, 2 * N_DIRS * N_CHUNKS], F32, kind="ExternalOutput"
    ).ap()
    counts_out = nc.dram_tensor("counts", [N_DIRS, L], F32, kind="ExternalOutput").ap()
    from contextlib import ExitStack

    with tile.TileContext(nc) as tc:
        with ExitStack() as ctx:
            _body(ctx, tc, rgb, ir, stats_out, counts_out)
    nc.compile()
    _CACHE["nc"] = nc
    return nc


def _combine(per_core):
    """per_core: list of (stats[128, 128], counts[8, 1024]) -> scalar loss."""
    ncols = N_DIRS * N_CHUNKS
    total = 0.0
    for stats, counts in per_core:
        lmax = stats[:, 0:ncols].astype(np.float64)
        lsum = stats[:, ncols : 2 * ncols].astype(np.float64)
        cnts = counts.astype(np.float64)
        for m in range(M_PER_CORE):
            dS, dT = 2 * m, 2 * m + 1
            lmax_S = lmax[:, dS * N_CHUNKS : (dS + 1) * N_CHUNKS].T.ravel()
            lmax_T = lmax[:, dT * N_CHUNKS : (dT + 1) * N_CHUNKS].T.ravel()
            lsum_S = lsum[:, dS * N_CHUNKS : (dS + 1) * N_CHUNKS]
            lsum_T = lsum[:, dT * N_CHUNKS : (dT + 1) * N_CHUNKS]
            total += (
                lmax_S.sum()
                + lmax_T.sum()
                + cnts[dS] @ lmax_T
                + cnts[dT] @ lmax_S
                - 2.0 * (lsum_S.sum() + lsum_T.sum())
            )
    return np.float32(-total / (2 * N * L))


def run(rgb_map, ir_map, trace=False):
    nc = _build()
    rgb = np.ascontiguousarray(np.asarray(rgb_map, np.float32).reshape(N, C, L))
    ir = np.ascontiguousarray(np.asarray(ir_map, np.float32).reshape(N, C, L))
    in_maps = [
        {
            "rgb": rgb[c * M_PER_CORE : (c + 1) * M_PER_CORE],
            "ir": ir[c * M_PER_CORE : (c + 1) * M_PER_CORE],
        }
        for c in range(N_CORES)
    ]
    res = run_bass_kernel_spmd(nc, in_maps, list(range(N_CORES)), trace=trace)
    per_core = [(r["stats"], r["counts"]) for r in res.results]
    return _combine(per_core), res


def kernel(rgb_map, ir_map):
    out, _ = run(rgb_map, ir_map)
    return out


# revision 33
# speedup vs baseline: 1.0392x; 1.0392x over previous
"""PixCycleContrastive loss kernel for 8 Trainium2 NeuronCores.

Data-parallel over N=32 images (4 per core). Per image and direction the
[1024,1024] logit block is computed on PE (fp16 inputs, fp32 accumulate),
exp+row-sum on ACT (accum_out), row-max on DVE (fp32), argmax-onehot mask on
GPSIMD, and the "gathered colmax" terms are reduced to per-column counts via a
ones-matmul on PE (sum_i log max1[arg2[i]] == sum_j count2[j] * log max1[j]).
Device returns per-row log-stats + counts; the host does the final (tiny)
linear combine == the scalar all-reduce.
"""

import numpy as np

import concourse.bacc as bacc
import concourse.bass as bass
import concourse.mybir as mybir
import concourse.tile as tile
from concourse.bass_utils import run_bass_kernel_spmd

F32 = mybir.dt.float32
F16 = mybir.dt.float16
AX = mybir.AxisListType
AF = mybir.ActivationFunctionType
ALU = mybir.AluOpType

N_CORES = 8
N, C, H, W = 32, 64, 32, 32
L = H * W  # 1024
M_PER_CORE = N // N_CORES  # 4 images per core
N_TENS = 2 * M_PER_CORE  # 8 input maps per core (rgb/ir x 4 images)
N_DIRS = 2 * M_PER_CORE  # 8 image-directions per core
CHUNK = 128
N_CHUNKS = L // CHUNK  # 8
TEMP = 0.1
EPS = 1e-6


def _body(ctx, tc, rgb, ir, stats_out, counts_out):
    nc = tc.nc

    from contextlib import ExitStack

    const_pool = ctx.enter_context(tc.tile_pool(name="const", bufs=1))
    xn_pool = ctx.enter_context(tc.tile_pool(name="xn", bufs=1))
    stats_pool = ctx.enter_context(tc.tile_pool(name="stats", bufs=1))
    e_pool = ctx.enter_context(tc.tile_pool(name="e", bufs=3))
    mask_pool = ctx.enter_context(tc.tile_pool(name="mask", bufs=5))
    scr_pool = ctx.enter_context(tc.tile_pool(name="scr", bufs=2))
    csb_pool = ctx.enter_context(tc.tile_pool(name="csb", bufs=2))

    ones64 = const_pool.tile([C, CHUNK], F16, tag="ones64")
    nc.vector.memset(ones64[:], 1.0)
    ones128 = const_pool.tile([CHUNK, 1], F16, tag="ones128")
    nc.vector.memset(ones128[:], 1.0)
    eps_tile = const_pool.tile([CHUNK, 1], F32, tag="eps")
    nc.vector.memset(eps_tile[:], EPS)

    # Normalized fp16 inputs, all resident, one tile per map so the main loop
    # can start as soon as its two maps are ready. t=2m+{0:rgb,1:ir}.
    xn = [xn_pool.tile([C, L], F16, tag=f"xn{t}", name=f"xn{t}") for t in range(N_TENS)]

    # Main-loop PSUM pools FIRST so they own banks 0-5 and the main loop can
    # start while the norm stage still holds banks 6-7.
    s_psum_pool = ctx.enter_context(tc.tile_pool(name="spsum", bufs=2, space="PSUM"))
    c_psum_pool = ctx.enter_context(tc.tile_pool(name="cpsum", bufs=1, space="PSUM"))

    # ---- Stage 0: load + l2-normalize (over C) each [64, 1024] map ----
    with ExitStack() as norm_ctx:
        x_pool = norm_ctx.enter_context(tc.tile_pool(name="x", bufs=4))
        xsq_pool = norm_ctx.enter_context(tc.tile_pool(name="xsq", bufs=4))
        rn_pool = norm_ctx.enter_context(tc.tile_pool(name="rn", bufs=4))
        norm_psum = norm_ctx.enter_context(
            tc.tile_pool(name="npsum", bufs=1, space="PSUM")
        )
        for t in range(N_TENS):
            m, is_ir = divmod(t, 2)
            src = (ir if is_ir else rgb)[m]
            x = x_pool.tile([C, L], F32, tag="x")
            nc.sync.dma_start(x[:], src)
            xsq = xsq_pool.tile([C, L], F16, tag="xsq")
            nc.gpsimd.tensor_mul(xsq[:], x[:], x[:])
            # nsq replicated across all 128 output partitions (M=128 ones)
            nsq = norm_psum.tile([C, L], F32, tag="nsq")
            for h in range(2):
                nc.tensor.matmul(
                    nsq[:, h * 512 : (h + 1) * 512],
                    ones64[:, :C],
                    xsq[:, h * 512 : (h + 1) * 512],
                    start=True,
                    stop=True,
                )
            rlog = rn_pool.tile([C, L], F32, tag="rlog")
            nc.scalar.activation(rlog[:], nsq[:], AF.Ln)
            rn = rn_pool.tile([C, L], F32, tag="rn")
            nc.scalar.activation(rn[:], rlog[:], AF.Exp, scale=-0.5)
            nc.gpsimd.tensor_mul(xn[t][:], x[:], rn[:])

    # ---- Stage 1: per image-direction logit chunks + reductions ----
    # dir d = 2m + s: s=0 rows=rgb (lhsT=rgbn, rhs=irn), s=1 rows=ir.
    emax_all = stats_pool.tile([CHUNK, N_DIRS * N_CHUNKS], F32, tag="emax")
    sum_all = stats_pool.tile([CHUNK, N_DIRS * N_CHUNKS], F32, tag="sum")

    for d in range(N_DIRS):
        m, s = divmod(d, 2)
        lhs_t = 2 * m + s  # rgb for s=0, ir for s=1
        rhs_t = 2 * m + (1 - s)
        lhs = xn[lhs_t][:]
        rhs = xn[rhs_t][:]
        cnt = c_psum_pool.tile([97, L], F32, tag="cnt")
        masks = []
        for k in range(N_CHUNKS):
            sp = s_psum_pool.tile([CHUNK, L], F32, tag="sp")
            for h in range(2):
                nc.tensor.matmul(
                    sp[:, h * 512 : (h + 1) * 512],
                    lhs[:, k * CHUNK : (k + 1) * CHUNK],
                    rhs[:, h * 512 : (h + 1) * 512],
                    start=True,
                    stop=True,
                )
            col = d * N_CHUNKS + k
            e = e_pool.tile([CHUNK, L], F32, tag="e")
            nc.scalar.activation(
                e[:], sp[:], AF.Exp, scale=1.0 / TEMP,
                accum_out=sum_all[:, col : col + 1],
            )
            nc.vector.reduce_max(emax_all[:, col : col + 1], e[:], axis=AX.X)
            mask = mask_pool.tile([CHUNK, L], F16, tag="mask")
            nc.vector.tensor_scalar(
                mask[:], e[:], emax_all[:, col : col + 1], None, ALU.is_equal
            )
            masks.append(mask)
            # count-matmuls packed 4-wide via PE column tiling, two rounds
            if k % 4 == 3:
                r = k // 4
                for g in range(4):
                    for h in range(2):
                        nc.tensor.matmul(
                            cnt[32 * g : 32 * g + 1, h * 512 : (h + 1) * 512],
                            ones128[:],
                            masks[4 * r + g][:, h * 512 : (h + 1) * 512],
                            start=(r == 0),
                            stop=(r == 1),
                            tile_position=(0, 32 * g),
                        )
        cnt_sb = csb_pool.tile([97, L], F32, tag="cnt_sb")
        if d % 2 == 0:
            nc.scalar.copy(cnt_sb[:], cnt[:])
        else:
            nc.vector.tensor_copy(cnt_sb[:], cnt[:])
        for g in range(4):
            nc.sync.dma_start(counts_out[d, g], cnt_sb[32 * g : 32 * g + 1, :])

    # ---- Stage 2: logs of maxes and sums, ship stats ----
    logs = stats_pool.tile([CHUNK, 2 * N_DIRS * N_CHUNKS], F32, tag="logs")
    ncols = N_DIRS * N_CHUNKS  # 64
    nc.scalar.activation(logs[:, 0:ncols], emax_all[:], AF.Ln)
    nc.scalar.activation(logs[:, ncols : 2 * ncols], sum_all[:], AF.Ln, bias=eps_tile[:])
    nc.sync.dma_start(stats_out, logs[:])


_CACHE = {}

# Restrict the ACT table sets the compiler may pick so Exp and Ln resolve to
# the SAME set (natural_log_exp_and_others) -> no table reloads mid-kernel.
_KEEP_SETS = {"natural_log_exp_and_others"}


def _patched_tables(module_arch):
    from concourse.hw_specs import get_activation_tables

    t = get_activation_tables(module_arch)
    return {name: (funcs if name in _KEEP_SETS else set()) for name, funcs in t.items()}


def _build():
    if "nc" in _CACHE:
        return _CACHE["nc"]
    bacc.get_activation_tables = _patched_tables
    nc = bacc.Bacc(
        "TRN2", target_bir_lowering=False, debug=False, num_devices=N_CORES
    )
    rgb = nc.dram_tensor("rgb", [M_PER_CORE, C, L], F32, kind="ExternalInput").ap()
    ir = nc.dram_tensor("ir", [M_PER_CORE, C, L], F32, kind="ExternalInput").ap()
    stats_out = nc.dram_tensor(
        "stats", [CHUNK, 2 * N_DIRS * N_CHUNKS], F32, kind="ExternalOutput"
    ).ap()
    counts_out = nc.dram_tensor("counts", [N_DIRS, 4, L], F32, kind="ExternalOutput").ap()
    from contextlib import ExitStack

    with tile.TileContext(nc) as tc:
        with ExitStack() as ctx:
            _body(ctx, tc, rgb, ir, stats_out, counts_out)
    nc.compile()
    _CACHE["nc"] = nc
    return nc


def _combine(per_core):
    """per_core: list of (stats[128, 128], counts[8, 1024]) -> scalar loss."""
    ncols = N_DIRS * N_CHUNKS
    total = 0.0
    for stats, counts in per_core:
        lmax = stats[:, 0:ncols].astype(np.float64)
        lsum = stats[:, ncols : 2 * ncols].astype(np.float64)
        cnts = counts.astype(np.float64).sum(1)
        for m in range(M_PER_CORE):
            dS, dT = 2 * m, 2 * m + 1
            lmax_S = lmax[:, dS * N_CHUNKS : (dS + 1) * N_CHUNKS].T.ravel()
            lmax_T = lmax[:, dT * N_CHUNKS : (dT + 1) * N_CHUNKS].T.ravel()
            lsum_S = lsum[:, dS * N_CHUNKS : (dS + 1) * N_CHUNKS]
            lsum_T = lsum[:, dT * N_CHUNKS : (dT + 1) * N_CHUNKS]
            total += (
                lmax_S.sum()
                + lmax_T.sum()
                + cnts[dS] @ lmax_T
                + cnts[dT] @ lmax_S
                - 2.0 * (lsum_S.sum() + lsum_T.sum())
            )
    return np.float32(-total / (2 * N * L))


def run(rgb_map, ir_map, trace=False):
    nc = _build()
    rgb = np.ascontiguousarray(np.asarray(rgb_map, np.float32).reshape(N, C, L))
    ir = np.ascontiguousarray(np.asarray(ir_map, np.float32).reshape(N, C, L))
    in_maps = [
        {
            "rgb": rgb[c * M_PER_CORE : (c + 1) * M_PER_CORE],
            "ir": ir[c * M_PER_CORE : (c + 1) * M_PER_CORE],
        }
        for c in range(N_CORES)
    ]
    res = run_bass_kernel_spmd(nc, in_maps, list(range(N_CORES)), trace=trace)
    per_core = [(r["stats"], r["counts"]) for r in res.results]
    return _combine(per_core), res


def kernel(rgb_map, ir_map):
    out, _ = run(rgb_map, ir_map)
    return out


# revision 34
# speedup vs baseline: 1.0402x; 1.0010x over previous
"""PixCycleContrastive loss kernel for 8 Trainium2 NeuronCores.

Data-parallel over N=32 images (4 per core). Per image and direction the
[1024,1024] logit block is computed on PE (fp16 inputs, fp32 accumulate),
exp+row-sum on ACT (accum_out), row-max on DVE (fp32), argmax-onehot mask on
GPSIMD, and the "gathered colmax" terms are reduced to per-column counts via a
ones-matmul on PE (sum_i log max1[arg2[i]] == sum_j count2[j] * log max1[j]).
Device returns per-row log-stats + counts; the host does the final (tiny)
linear combine == the scalar all-reduce.
"""

import numpy as np

import concourse.bacc as bacc
import concourse.bass as bass
import concourse.mybir as mybir
import concourse.tile as tile
from concourse.bass_utils import run_bass_kernel_spmd

F32 = mybir.dt.float32
F16 = mybir.dt.float16
AX = mybir.AxisListType
AF = mybir.ActivationFunctionType
ALU = mybir.AluOpType

N_CORES = 8
N, C, H, W = 32, 64, 32, 32
L = H * W  # 1024
M_PER_CORE = N // N_CORES  # 4 images per core
N_TENS = 2 * M_PER_CORE  # 8 input maps per core (rgb/ir x 4 images)
N_DIRS = 2 * M_PER_CORE  # 8 image-directions per core
CHUNK = 128
N_CHUNKS = L // CHUNK  # 8
TEMP = 0.1
EPS = 1e-6


def _body(ctx, tc, rgb, ir, stats_out, counts_out):
    nc = tc.nc

    from contextlib import ExitStack

    const_pool = ctx.enter_context(tc.tile_pool(name="const", bufs=1))
    xn_pool = ctx.enter_context(tc.tile_pool(name="xn", bufs=1))
    stats_pool = ctx.enter_context(tc.tile_pool(name="stats", bufs=1))
    e_pool = ctx.enter_context(tc.tile_pool(name="e", bufs=3))
    mask_pool = ctx.enter_context(tc.tile_pool(name="mask", bufs=5))
    scr_pool = ctx.enter_context(tc.tile_pool(name="scr", bufs=2))
    mx8_pool = ctx.enter_context(tc.tile_pool(name="mx8", bufs=2))
    csb_pool = ctx.enter_context(tc.tile_pool(name="csb", bufs=2))

    ones64 = const_pool.tile([C, CHUNK], F16, tag="ones64")
    nc.vector.memset(ones64[:], 1.0)
    ones128 = const_pool.tile([CHUNK, 1], F16, tag="ones128")
    nc.vector.memset(ones128[:], 1.0)
    eps_tile = const_pool.tile([CHUNK, 1], F32, tag="eps")
    nc.vector.memset(eps_tile[:], EPS)

    # Normalized fp16 inputs, all resident, one tile per map so the main loop
    # can start as soon as its two maps are ready. t=2m+{0:rgb,1:ir}.
    xn = [xn_pool.tile([C, L], F16, tag=f"xn{t}", name=f"xn{t}") for t in range(N_TENS)]

    # Main-loop PSUM pools FIRST so they own banks 0-5 and the main loop can
    # start while the norm stage still holds banks 6-7.
    s_psum_pool = ctx.enter_context(tc.tile_pool(name="spsum", bufs=2, space="PSUM"))
    c_psum_pool = ctx.enter_context(tc.tile_pool(name="cpsum", bufs=1, space="PSUM"))

    # ---- Stage 0: load + l2-normalize (over C) each [64, 1024] map ----
    with ExitStack() as norm_ctx:
        x_pool = norm_ctx.enter_context(tc.tile_pool(name="x", bufs=4))
        xsq_pool = norm_ctx.enter_context(tc.tile_pool(name="xsq", bufs=4))
        rn_pool = norm_ctx.enter_context(tc.tile_pool(name="rn", bufs=4))
        norm_psum = norm_ctx.enter_context(
            tc.tile_pool(name="npsum", bufs=1, space="PSUM")
        )
        for t in range(N_TENS):
            m, is_ir = divmod(t, 2)
            src = (ir if is_ir else rgb)[m]
            x = x_pool.tile([C, L], F32, tag="x")
            nc.sync.dma_start(x[:], src)
            xsq = xsq_pool.tile([C, L], F16, tag="xsq")
            nc.gpsimd.tensor_mul(xsq[:], x[:], x[:])
            # nsq replicated across all 128 output partitions (M=128 ones)
            nsq = norm_psum.tile([C, L], F32, tag="nsq")
            for h in range(2):
                nc.tensor.matmul(
                    nsq[:, h * 512 : (h + 1) * 512],
                    ones64[:, :C],
                    xsq[:, h * 512 : (h + 1) * 512],
                    start=True,
                    stop=True,
                )
            rlog = rn_pool.tile([C, L], F32, tag="rlog")
            nc.scalar.activation(rlog[:], nsq[:], AF.Ln)
            rn = rn_pool.tile([C, L], F32, tag="rn")
            nc.scalar.activation(rn[:], rlog[:], AF.Exp, scale=-0.5)
            nc.gpsimd.tensor_mul(xn[t][:], x[:], rn[:])

    # ---- Stage 1: per image-direction logit chunks + reductions ----
    # dir d = 2m + s: s=0 rows=rgb (lhsT=rgbn, rhs=irn), s=1 rows=ir.
    emax_all = stats_pool.tile([CHUNK, N_DIRS * N_CHUNKS], F32, tag="emax")
    sum_all = stats_pool.tile([CHUNK, N_DIRS * N_CHUNKS], F32, tag="sum")

    for d in range(N_DIRS):
        m, s = divmod(d, 2)
        lhs_t = 2 * m + s  # rgb for s=0, ir for s=1
        rhs_t = 2 * m + (1 - s)
        lhs = xn[lhs_t][:]
        rhs = xn[rhs_t][:]
        cnt = c_psum_pool.tile([97, L], F32, tag="cnt")
        masks = []
        for k in range(N_CHUNKS):
            sp = s_psum_pool.tile([CHUNK, L], F32, tag="sp")
            for h in range(2):
                nc.tensor.matmul(
                    sp[:, h * 512 : (h + 1) * 512],
                    lhs[:, k * CHUNK : (k + 1) * CHUNK],
                    rhs[:, h * 512 : (h + 1) * 512],
                    start=True,
                    stop=True,
                )
            col = d * N_CHUNKS + k
            e = e_pool.tile([CHUNK, L], F32, tag="e")
            nc.scalar.activation(
                e[:], sp[:], AF.Exp, scale=1.0 / TEMP,
                accum_out=sum_all[:, col : col + 1],
            )
            if k % 2 == 0:
                mx8 = mx8_pool.tile([CHUNK, 8], F32, tag="mx8")
                nc.vector.max(mx8[:], e[:])
                nc.vector.tensor_copy(emax_all[:, col : col + 1], mx8[:, 0:1])
                emax_src = mx8[:, 0:1]
            else:
                nc.vector.reduce_max(emax_all[:, col : col + 1], e[:], axis=AX.X)
                emax_src = emax_all[:, col : col + 1]
            mask = mask_pool.tile([CHUNK, L], F16, tag="mask")
            nc.vector.tensor_scalar(
                mask[:], e[:], emax_src, None, ALU.is_equal
            )
            masks.append(mask)
            # count-matmuls packed 4-wide via PE column tiling, two rounds
            if k % 4 == 3:
                r = k // 4
                for g in range(4):
                    for h in range(2):
                        nc.tensor.matmul(
                            cnt[32 * g : 32 * g + 1, h * 512 : (h + 1) * 512],
                            ones128[:],
                            masks[4 * r + g][:, h * 512 : (h + 1) * 512],
                            start=(r == 0),
                            stop=(r == 1),
                            tile_position=(0, 32 * g),
                        )
        cnt_sb = csb_pool.tile([97, L], F32, tag="cnt_sb")
        if d % 2 == 0:
            nc.scalar.copy(cnt_sb[:], cnt[:])
        else:
            nc.vector.tensor_copy(cnt_sb[:], cnt[:])
        for g in range(4):
            nc.sync.dma_start(counts_out[d, g], cnt_sb[32 * g : 32 * g + 1, :])

    # ---- Stage 2: logs of maxes and sums, ship stats ----
    logs = stats_pool.tile([CHUNK, 2 * N_DIRS * N_CHUNKS], F32, tag="logs")
    ncols = N_DIRS * N_CHUNKS  # 64
    nc.scalar.activation(logs[:, 0:ncols], emax_all[:], AF.Ln)
    nc.scalar.activation(logs[:, ncols : 2 * ncols], sum_all[:], AF.Ln, bias=eps_tile[:])
    nc.sync.dma_start(stats_out, logs[:])


_CACHE = {}

# Restrict the ACT table sets the compiler may pick so Exp and Ln resolve to
# the SAME set (natural_log_exp_and_others) -> no table reloads mid-kernel.
_KEEP_SETS = {"natural_log_exp_and_others"}


def _patched_tables(module_arch):
    from concourse.hw_specs import get_activation_tables

    t = get_activation_tables(module_arch)
    return {name: (funcs if name in _KEEP_SETS else set()) for name, funcs in t.items()}


def _build():
    if "nc" in _CACHE:
        return _CACHE["nc"]
    bacc.get_activation_tables = _patched_tables
    nc = bacc.Bacc(
        "TRN2", target_bir_lowering=False, debug=False, num_devices=N_CORES
    )
    rgb = nc.dram_tensor("rgb", [M_PER_CORE, C, L], F32, kind="ExternalInput").ap()
    ir = nc.dram_tensor("ir", [M_PER_CORE, C, L], F32, kind="ExternalInput").ap()
    stats_out = nc.dram_tensor(
        "stats", [CHUNK, 2 * N_DIRS * N_CHUNKS], F32, kind="ExternalOutput"
    ).ap()
    counts_out = nc.dram_tensor("counts", [N_DIRS, 4, L], F32, kind="ExternalOutput").ap()
    from contextlib import ExitStack

    with tile.TileContext(nc) as tc:
        with ExitStack() as ctx:
            _body(ctx, tc, rgb, ir, stats_out, counts_out)
    nc.compile()
    _CACHE["nc"] = nc
    return nc


def _combine(per_core):
    """per_core: list of (stats[128, 128], counts[8, 1024]) -> scalar loss."""
    ncols = N_DIRS * N_CHUNKS
    total = 0.0
    for stats, counts in per_core:
        lmax = stats[:, 0:ncols].astype(np.float64)
        lsum = stats[:, ncols : 2 * ncols].astype(np.float64)
        cnts = counts.astype(np.float64).sum(1)
        for m in range(M_PER_CORE):
            dS, dT = 2 * m, 2 * m + 1
            lmax_S = lmax[:, dS * N_CHUNKS : (dS + 1) * N_CHUNKS].T.ravel()
            lmax_T = lmax[:, dT * N_CHUNKS : (dT + 1) * N_CHUNKS].T.ravel()
            lsum_S = lsum[:, dS * N_CHUNKS : (dS + 1) * N_CHUNKS]
            lsum_T = lsum[:, dT * N_CHUNKS : (dT + 1) * N_CHUNKS]
            total += (
                lmax_S.sum()
                + lmax_T.sum()
                + cnts[dS] @ lmax_T
                + cnts[dT] @ lmax_S
                - 2.0 * (lsum_S.sum() + lsum_T.sum())
            )
    return np.float32(-total / (2 * N * L))


def run(rgb_map, ir_map, trace=False):
    nc = _build()
    rgb = np.ascontiguousarray(np.asarray(rgb_map, np.float32).reshape(N, C, L))
    ir = np.ascontiguousarray(np.asarray(ir_map, np.float32).reshape(N, C, L))
    in_maps = [
        {
            "rgb": rgb[c * M_PER_CORE : (c + 1) * M_PER_CORE],
            "ir": ir[c * M_PER_CORE : (c + 1) * M_PER_CORE],
        }
        for c in range(N_CORES)
    ]
    res = run_bass_kernel_spmd(nc, in_maps, list(range(N_CORES)), trace=trace)
    per_core = [(r["stats"], r["counts"]) for r in res.results]
    return _combine(per_core), res


def kernel(rgb_map, ir_map):
    out, _ = run(rgb_map, ir_map)
    return out


# revision 35
# speedup vs baseline: 1.0496x; 1.0089x over previous
"""PixCycleContrastive loss kernel for 8 Trainium2 NeuronCores.

Data-parallel over N=32 images (4 per core). Per image and direction the
[1024,1024] logit block is computed on PE (fp16 inputs, fp32 accumulate),
exp+row-sum on ACT (accum_out), row-max on DVE (fp32), argmax-onehot mask on
GPSIMD, and the "gathered colmax" terms are reduced to per-column counts via a
ones-matmul on PE (sum_i log max1[arg2[i]] == sum_j count2[j] * log max1[j]).
Device returns per-row log-stats + counts; the host does the final (tiny)
linear combine == the scalar all-reduce.
"""

import numpy as np

import concourse.bacc as bacc
import concourse.bass as bass
import concourse.mybir as mybir
import concourse.tile as tile
from concourse.bass_utils import run_bass_kernel_spmd

F32 = mybir.dt.float32
F16 = mybir.dt.float16
AX = mybir.AxisListType
AF = mybir.ActivationFunctionType
ALU = mybir.AluOpType

N_CORES = 8
N, C, H, W = 32, 64, 32, 32
L = H * W  # 1024
M_PER_CORE = N // N_CORES  # 4 images per core
N_TENS = 2 * M_PER_CORE  # 8 input maps per core (rgb/ir x 4 images)
N_DIRS = 2 * M_PER_CORE  # 8 image-directions per core
CHUNK = 128
N_CHUNKS = L // CHUNK  # 8
TEMP = 0.1
EPS = 1e-6


def _body(ctx, tc, rgb, ir, stats_out, counts_out):
    nc = tc.nc

    from contextlib import ExitStack

    const_pool = ctx.enter_context(tc.tile_pool(name="const", bufs=1))
    xn_pool = ctx.enter_context(tc.tile_pool(name="xn", bufs=1))
    stats_pool = ctx.enter_context(tc.tile_pool(name="stats", bufs=1))
    e_pool = ctx.enter_context(tc.tile_pool(name="e", bufs=4))
    mask_pool = ctx.enter_context(tc.tile_pool(name="mask", bufs=5))
    scr_pool = ctx.enter_context(tc.tile_pool(name="scr", bufs=2))
    csb_pool = ctx.enter_context(tc.tile_pool(name="csb", bufs=2))

    ones64 = const_pool.tile([C, CHUNK], F16, tag="ones64")
    nc.vector.memset(ones64[:], 1.0)
    ones128 = const_pool.tile([CHUNK, 1], F16, tag="ones128")
    nc.vector.memset(ones128[:], 1.0)
    eps_tile = const_pool.tile([CHUNK, 1], F32, tag="eps")
    nc.vector.memset(eps_tile[:], EPS)

    # Normalized fp16 inputs, all resident, one tile per map so the main loop
    # can start as soon as its two maps are ready. t=2m+{0:rgb,1:ir}.
    xn = [xn_pool.tile([C, L], F16, tag=f"xn{t}", name=f"xn{t}") for t in range(N_TENS)]

    # Main-loop PSUM pools FIRST so they own banks 0-5 and the main loop can
    # start while the norm stage still holds banks 6-7.
    s_psum_pool = ctx.enter_context(tc.tile_pool(name="spsum", bufs=2, space="PSUM"))
    c_psum_pool = ctx.enter_context(tc.tile_pool(name="cpsum", bufs=1, space="PSUM"))

    # ---- Stage 0: load + l2-normalize (over C) each [64, 1024] map ----
    with ExitStack() as norm_ctx:
        x_pool = norm_ctx.enter_context(tc.tile_pool(name="x", bufs=4))
        xsq_pool = norm_ctx.enter_context(tc.tile_pool(name="xsq", bufs=4))
        rn_pool = norm_ctx.enter_context(tc.tile_pool(name="rn", bufs=4))
        norm_psum = norm_ctx.enter_context(
            tc.tile_pool(name="npsum", bufs=1, space="PSUM")
        )
        for t in range(N_TENS):
            m, is_ir = divmod(t, 2)
            src = (ir if is_ir else rgb)[m]
            x = x_pool.tile([C, L], F32, tag="x")
            nc.sync.dma_start(x[:], src)
            xsq = xsq_pool.tile([C, L], F16, tag="xsq")
            nc.gpsimd.tensor_mul(xsq[:], x[:], x[:])
            # nsq replicated across all 128 output partitions (M=128 ones)
            nsq = norm_psum.tile([C, L], F32, tag="nsq")
            for h in range(2):
                nc.tensor.matmul(
                    nsq[:, h * 512 : (h + 1) * 512],
                    ones64[:, :C],
                    xsq[:, h * 512 : (h + 1) * 512],
                    start=True,
                    stop=True,
                )
            rlog = rn_pool.tile([C, L], F32, tag="rlog")
            nc.scalar.activation(rlog[:], nsq[:], AF.Ln)
            rn = rn_pool.tile([C, L], F32, tag="rn")
            nc.scalar.activation(rn[:], rlog[:], AF.Exp, scale=-0.5)
            nc.gpsimd.tensor_mul(xn[t][:], x[:], rn[:])

    # ---- Stage 1: per image-direction logit chunks + reductions ----
    # dir d = 2m + s: s=0 rows=rgb (lhsT=rgbn, rhs=irn), s=1 rows=ir.
    emax_all = stats_pool.tile([CHUNK, N_DIRS * N_CHUNKS], F32, tag="emax")
    sum_all = stats_pool.tile([CHUNK, N_DIRS * N_CHUNKS], F32, tag="sum")
    logs = stats_pool.tile([CHUNK, 2 * N_DIRS * N_CHUNKS], F32, tag="logs")
    ncols = N_DIRS * N_CHUNKS  # 64

    for d in range(N_DIRS):
        m, s = divmod(d, 2)
        lhs_t = 2 * m + s  # rgb for s=0, ir for s=1
        rhs_t = 2 * m + (1 - s)
        lhs = xn[lhs_t][:]
        rhs = xn[rhs_t][:]
        cnt = c_psum_pool.tile([97, L], F32, tag="cnt")
        masks = []
        for k in range(N_CHUNKS):
            sp = s_psum_pool.tile([CHUNK, L], F32, tag="sp")
            for h in range(2):
                nc.tensor.matmul(
                    sp[:, h * 512 : (h + 1) * 512],
                    lhs[:, k * CHUNK : (k + 1) * CHUNK],
                    rhs[:, h * 512 : (h + 1) * 512],
                    start=True,
                    stop=True,
                )
            col = d * N_CHUNKS + k
            e = e_pool.tile([CHUNK, L], F32, tag="e")
            nc.scalar.activation(
                e[:], sp[:], AF.Exp, scale=1.0 / TEMP,
                accum_out=sum_all[:, col : col + 1],
            )
            nc.vector.reduce_max(emax_all[:, col : col + 1], e[:], axis=AX.X)
            mask = mask_pool.tile([CHUNK, L], F16, tag="mask")
            nc.vector.tensor_scalar(
                mask[:], e[:], emax_all[:, col : col + 1], None, ALU.is_equal
            )
            masks.append(mask)
            # count-matmuls packed 4-wide via PE column tiling, two rounds
            if k % 4 == 3:
                r = k // 4
                for g in range(4):
                    for h in range(2):
                        nc.tensor.matmul(
                            cnt[32 * g : 32 * g + 1, h * 512 : (h + 1) * 512],
                            ones128[:],
                            masks[4 * r + g][:, h * 512 : (h + 1) * 512],
                            start=(r == 0),
                            stop=(r == 1),
                            tile_position=(0, 32 * g),
                        )
        nc.scalar.activation(
            logs[:, d * N_CHUNKS : (d + 1) * N_CHUNKS],
            emax_all[:, d * N_CHUNKS : (d + 1) * N_CHUNKS], AF.Ln)
        nc.scalar.activation(
            logs[:, ncols + d * N_CHUNKS : ncols + (d + 1) * N_CHUNKS],
            sum_all[:, d * N_CHUNKS : (d + 1) * N_CHUNKS], AF.Ln, bias=eps_tile[:])
        cnt_sb = csb_pool.tile([97, L], F32, tag="cnt_sb")
        if d % 2 == 0:
            nc.scalar.copy(cnt_sb[:], cnt[:])
        else:
            nc.vector.tensor_copy(cnt_sb[:], cnt[:])
        for g in range(4):
            nc.sync.dma_start(counts_out[d, g], cnt_sb[32 * g : 32 * g + 1, :])

    nc.sync.dma_start(stats_out, logs[:])


_CACHE = {}

# Restrict the ACT table sets the compiler may pick so Exp and Ln resolve to
# the SAME set (natural_log_exp_and_others) -> no table reloads mid-kernel.
_KEEP_SETS = {"natural_log_exp_and_others"}


def _patched_tables(module_arch):
    from concourse.hw_specs import get_activation_tables

    t = get_activation_tables(module_arch)
    return {name: (funcs if name in _KEEP_SETS else set()) for name, funcs in t.items()}


def _build():
    if "nc" in _CACHE:
        return _CACHE["nc"]
    bacc.get_activation_tables = _patched_tables
    nc = bacc.Bacc(
        "TRN2", target_bir_lowering=False, debug=False, num_devices=N_CORES
    )
    rgb = nc.dram_tensor("rgb", [M_PER_CORE, C, L], F32, kind="ExternalInput").ap()
    ir = nc.dram_tensor("ir", [M_PER_CORE, C, L], F32, kind="ExternalInput").ap()
    stats_out = nc.dram_tensor(
        "stats", [CHUNK, 2 * N_DIRS * N_CHUNKS], F32, kind="ExternalOutput"
    ).ap()
    counts_out = nc.dram_tensor("counts", [N_DIRS, 4, L], F32, kind="ExternalOutput").ap()
    from contextlib import ExitStack

    with tile.TileContext(nc) as tc:
        with ExitStack() as ctx:
            _body(ctx, tc, rgb, ir, stats_out, counts_out)
    nc.compile()
    _CACHE["nc"] = nc
    return nc


def _combine(per_core):
    """per_core: list of (stats[128, 128], counts[8, 1024]) -> scalar loss."""
    ncols = N_DIRS * N_CHUNKS
    total = 0.0
    for stats, counts in per_core:
        lmax = stats[:, 0:ncols].astype(np.float64)
        lsum = stats[:, ncols : 2 * ncols].astype(np.float64)
        cnts = counts.astype(np.float64).sum(1)
        for m in range(M_PER_CORE):
            dS, dT = 2 * m, 2 * m + 1
            lmax_S = lmax[:, dS * N_CHUNKS : (dS + 1) * N_CHUNKS].T.ravel()
            lmax_T = lmax[:, dT * N_CHUNKS : (dT + 1) * N_CHUNKS].T.ravel()
            lsum_S = lsum[:, dS * N_CHUNKS : (dS + 1) * N_CHUNKS]
            lsum_T = lsum[:, dT * N_CHUNKS : (dT + 1) * N_CHUNKS]
            total += (
                lmax_S.sum()
                + lmax_T.sum()
                + cnts[dS] @ lmax_T
                + cnts[dT] @ lmax_S
                - 2.0 * (lsum_S.sum() + lsum_T.sum())
            )
    return np.float32(-total / (2 * N * L))


def run(rgb_map, ir_map, trace=False):
    nc = _build()
    rgb = np.ascontiguousarray(np.asarray(rgb_map, np.float32).reshape(N, C, L))
    ir = np.ascontiguousarray(np.asarray(ir_map, np.float32).reshape(N, C, L))
    in_maps = [
        {
            "rgb": rgb[c * M_PER_CORE : (c + 1) * M_PER_CORE],
            "ir": ir[c * M_PER_CORE : (c + 1) * M_PER_CORE],
        }
        for c in range(N_CORES)
    ]
    res = run_bass_kernel_spmd(nc, in_maps, list(range(N_CORES)), trace=trace)
    per_core = [(r["stats"], r["counts"]) for r in res.results]
    return _combine(per_core), res


def kernel(rgb_map, ir_map):
    out, _ = run(rgb_map, ir_map)
    return out
